# revision 7
# baseline (speedup 1.0000x reference)
"""Trainium2 Bass kernel for nn_GroupBy (dual scatter-add segment sum).

reference:
    out = zeros([N, 128]); out[i1] += deltas[:, :128]; out[i2] += deltas[:, 128:256]
    b   = deltas[:, 256:320]
    return out, b

Strategy (8 NeuronCores):
  - Shard output rows across cores (12544 rows per core).
  - Host-side shard step: form 2E (target, source-row) pairs, order each
    core's pairs by target window (W_ROWS rows), pad windows to a shared
    static SPMD schedule.  Index math + one permutation pass only; every
    floating-point += happens on device.
  - Device: stream pair payload sequentially (full-BW DMAs), build one-hot
    selection matrices (DVE is_equal vs a static iota, OH_BATCH tiles per
    op), matmul-accumulate into PSUM per window (TensorE, fp32 PSUM).
    Copy finished windows into an SBUF staging buffer; one final DMA writes
    the core's output slab.
  - Host unshard: transpose window slabs back to row-major and concatenate.
"""
import os
import numpy as np

from concourse import bass, bacc, tile, mybir
from concourse.bass_utils import run_bass_kernel_spmd

N = 100000
E = 500000
FEAT = 128
P = 128
N_CORES = 8
ROWS_PER_CORE = 12544
W_ROWS = int(os.environ.get("GROUPBY_WROWS", "64"))      # rows per window
WIN_PER_CORE = ROWS_PER_CORE // W_ROWS
W_PACK = P // W_ROWS            # windows packed per 128 staging partitions
CHUNK_TILES = 64                # payload tiles (128 pairs each) per DMA chunk
OH_BATCH = 8                    # one-hot tiles built per DVE op
DTYPE = os.environ.get("GROUPBY_DTYPE", "fp16")


def _plan(i1, i2):
    """Host shard step: order (target, source) pairs by target window per core.

    Pure index math on int arrays.
    """
    t_all = np.concatenate([i1[:, 0], i2[:, 0]]).astype(np.int64)
    pair_ids = np.arange(2 * E, dtype=np.int64)
    order = np.argsort(t_all, kind="stable")
    t_sorted = t_all[order]
    ids_sorted = pair_ids[order]

    core_of = t_sorted // ROWS_PER_CORE          # contiguous runs, 0..7
    core_starts = np.searchsorted(core_of, np.arange(N_CORES + 1))

    per_core = []
    counts = np.zeros((N_CORES, WIN_PER_CORE), dtype=np.int64)
    for k in range(N_CORES):
        s, e = core_starts[k], core_starts[k + 1]
        t_loc = t_sorted[s:e] - k * ROWS_PER_CORE
        w = t_loc // W_ROWS
        r = t_loc % W_ROWS
        counts[k] = np.bincount(w, minlength=WIN_PER_CORE)
        per_core.append((w, r, ids_sorted[s:e]))

    t_w = np.maximum(1, (counts.max(axis=0) + P - 1) // P)  # tiles per window
    n_tiles = int(t_w.sum())
    lcm = np.lcm(CHUNK_TILES, OH_BATCH)
    n_tiles = int(((n_tiles + lcm - 1) // lcm) * lcm)
    slot_base = np.zeros(WIN_PER_CORE + 1, dtype=np.int64)
    slot_base[1:] = np.cumsum(t_w * P)

    cores = []
    for k in range(N_CORES):
        w, r, ids = per_core[k]
        win_start = np.searchsorted(w, np.arange(WIN_PER_CORE))
        idx_in_win = np.arange(len(w)) - win_start[w]
        slots = slot_base[w] + idx_in_win
        rvals = np.full((n_tiles * P,), -1.0, dtype=np.float32)
        rvals[slots] = r.astype(np.float32)
        cores.append((slots, ids, rvals))
    return n_tiles, t_w, cores


def _build_program(n_tiles, t_w, dt):
    nc = bacc.Bacc("TRN2", target_bir_lowering=False, debug=False)
    n_chunks = n_tiles // CHUNK_TILES
    payload = nc.dram_tensor(
        "payload", [n_chunks, P, CHUNK_TILES * FEAT], dt, kind="ExternalInput")
    rvals = nc.dram_tensor("rvals", [P, n_tiles], dt, kind="ExternalInput")
    iota = nc.dram_tensor("iota", [P, OH_BATCH * W_ROWS], dt, kind="ExternalInput")
    out = nc.dram_tensor(
        "out", [P, (WIN_PER_CORE // W_PACK) * FEAT], mybir.dt.float32,
        kind="ExternalOutput")

    with tile.TileContext(nc) as tc:
        with tc.tile_pool(name="sbuf", bufs=1) as sb, \
             tc.tile_pool(name="chunk", bufs=3) as chunkp, \
             tc.tile_pool(name="oh", bufs=4) as ohp, \
             tc.tile_pool(name="psum", bufs=8, space="PSUM") as pp:
            iota_t = sb.tile([P, OH_BATCH, W_ROWS], dt)
            nc.sync.dma_start(out=iota_t[:, :, :], in_=iota.ap())
            rv_t = sb.tile([P, n_tiles], dt)
            nc.sync.dma_start(out=rv_t[:], in_=rvals.ap())
            staging = sb.tile([P, (WIN_PER_CORE // W_PACK) * FEAT],
                              mybir.dt.float32)

            chunks = []
            for c in range(n_chunks):
                buf = chunkp.tile([P, CHUNK_TILES * FEAT], dt)
                nc.sync.dma_start(out=buf[:], in_=payload.ap()[c])
                chunks.append(buf)

            oh_blocks = [None] * (n_tiles // OH_BATCH)

            def oh_block(b):
                if oh_blocks[b] is None:
                    blk = ohp.tile([P, OH_BATCH, W_ROWS], dt)
                    nc.vector.tensor_tensor(
                        out=blk[:, :, :], in0=iota_t[:, :, :],
                        in1=rv_t[:, b * OH_BATCH:(b + 1) * OH_BATCH]
                            .to_broadcast([P, OH_BATCH, W_ROWS]),
                        op=mybir.AluOpType.is_equal)
                    oh_blocks[b] = blk
                return oh_blocks[b]

            t = 0
            for w in range(WIN_PER_CORE):
                ps = pp.tile([W_ROWS, FEAT], mybir.dt.float32, space="PSUM")
                tw = int(t_w[w])
                for j in range(tw):
                    c, tc_ = divmod(t, CHUNK_TILES)
                    blk = oh_block(t // OH_BATCH)
                    nc.tensor.matmul(
                        out=ps[:], lhsT=blk[:, t % OH_BATCH, :],
                        rhs=chunks[c][:, tc_ * FEAT:(tc_ + 1) * FEAT],
                        start=(j == 0), stop=(j == tw - 1))
                    t += 1
                h, w2 = w % W_PACK, w // W_PACK
                nc.scalar.copy(
                    out=staging[h * W_ROWS:(h + 1) * W_ROWS,
                                w2 * FEAT:(w2 + 1) * FEAT],
                    in_=ps[:])
                # flush finished staging quarters so the out-write overlaps
                n_w2 = WIN_PER_CORE // W_PACK
                if (w + 1) % (WIN_PER_CORE // 4) == 0:
                    q = (w + 1) // (WIN_PER_CORE // 4) - 1
                    c0 = q * (n_w2 // 4) * FEAT
                    c1 = (q + 1) * (n_w2 // 4) * FEAT if q < 3 else n_w2 * FEAT
                    nc.scalar.dma_start(
                        out=out.ap()[:, c0:c1], in_=staging[:, c0:c1])
    nc.compile()
    return nc


def kernel(unary, binary, deltas, index1, index2):
    dt = mybir.dt.float16 if DTYPE == "fp16" else mybir.dt.float32
    npdt = np.float16 if DTYPE == "fp16" else np.float32

    n_tiles, t_w, cores = _plan(np.asarray(index1), np.asarray(index2))
    n_chunks = n_tiles // CHUNK_TILES

    deltas = np.asarray(deltas)
    src = np.concatenate(
        [deltas[:, :FEAT], deltas[:, FEAT:2 * FEAT]], axis=0).astype(npdt)

    iota_np = np.tile(np.arange(W_ROWS, dtype=npdt), (P, OH_BATCH))
    in_maps = []
    for k in range(N_CORES):
        slots, ids, rvals = cores[k]
        pay = np.zeros((n_tiles * P, FEAT), dtype=npdt)
        pay[slots] = src[ids]
        pay = np.ascontiguousarray(
            pay.reshape(n_chunks, CHUNK_TILES, P, FEAT)
               .transpose(0, 2, 1, 3)
               .reshape(n_chunks, P, CHUNK_TILES * FEAT))
        in_maps.append({
            "payload": pay,
            "rvals": np.ascontiguousarray(rvals.reshape(n_tiles, P).T).astype(npdt),
            "iota": iota_np,
        })

    nc = _build_program(n_tiles, t_w, dt)

    trace = bool(int(os.environ.get("GROUPBY_TRACE", "0")))
    if trace:
        import trn_prof
        trn_prof.install()
    res = run_bass_kernel_spmd(nc, in_maps, list(range(N_CORES)), trace=trace)
    if trace:
        kernel.last_exec_time_ns = res.exec_time_ns

    out = np.empty((N_CORES * ROWS_PER_CORE, FEAT), dtype=np.float32)
    for k in range(N_CORES):
        slab = res.results[k]["out"]  # [P, (WIN//W_PACK)*FEAT]
        # staging[h*W_ROWS + r, w2*FEAT + f] = row (w2*W_PACK + h)*W_ROWS + r
        out[k * ROWS_PER_CORE:(k + 1) * ROWS_PER_CORE] = (
            slab.reshape(W_PACK, W_ROWS, WIN_PER_CORE // W_PACK, FEAT)
                .transpose(2, 0, 1, 3)
                .reshape(ROWS_PER_CORE, FEAT))
    out = out[:N]

    b = np.ascontiguousarray(deltas[:, 2 * FEAT:])
    return out, b


# revision 10
# speedup vs baseline: 1.1232x; 1.1232x over previous
"""Trainium2 Bass kernel for nn_GroupBy (dual scatter-add segment sum).

reference:
    out = zeros([N, 128]); out[i1] += deltas[:, :128]; out[i2] += deltas[:, 128:256]
    b   = deltas[:, 256:320]
    return out, b

Strategy (8 NeuronCores):
  - Shard output rows across cores (12544 rows per core).
  - Host-side shard step: form 2E (target, source-row) pairs, order each
    core's pairs by target window (W_ROWS rows), pad windows to a shared
    static SPMD schedule.  Index math + one permutation pass only; every
    floating-point += happens on device.
  - Device: stream pair payload sequentially (full-BW DMAs), build one-hot
    selection matrices (DVE is_equal vs a static iota, OH_BATCH tiles per
    op), matmul-accumulate into PSUM per window (TensorE, fp32 PSUM).
    Copy finished windows into an SBUF staging buffer; one final DMA writes
    the core's output slab.
  - Host unshard: transpose window slabs back to row-major and concatenate.
"""
import os
import numpy as np

from concourse import bass, bacc, tile, mybir
from concourse.bass_utils import run_bass_kernel_spmd

N = 100000
E = 500000
FEAT = 128
P = 128
N_CORES = 8
ROWS_PER_CORE = 12544
W_ROWS = int(os.environ.get("GROUPBY_WROWS", "64"))      # rows per window
WIN_PER_CORE = ROWS_PER_CORE // W_ROWS
W_PACK = P // W_ROWS            # windows packed per 128 staging partitions
CHUNK_TILES = 64                # payload tiles (128 pairs each) per DMA chunk
OH_BATCH = 8                    # one-hot tiles built per DVE op
DTYPE = os.environ.get("GROUPBY_DTYPE", "fp16")


def _plan(i1, i2):
    """Host shard step: order (target, source) pairs by target window per core.

    Pure index math on int arrays.
    """
    t_all = np.concatenate([i1[:, 0], i2[:, 0]]).astype(np.int64)
    pair_ids = np.arange(2 * E, dtype=np.int64)
    order = np.argsort(t_all, kind="stable")
    t_sorted = t_all[order]
    ids_sorted = pair_ids[order]

    core_of = t_sorted // ROWS_PER_CORE          # contiguous runs, 0..7
    core_starts = np.searchsorted(core_of, np.arange(N_CORES + 1))

    per_core = []
    counts = np.zeros((N_CORES, WIN_PER_CORE), dtype=np.int64)
    for k in range(N_CORES):
        s, e = core_starts[k], core_starts[k + 1]
        t_loc = t_sorted[s:e] - k * ROWS_PER_CORE
        w = t_loc // W_ROWS
        r = t_loc % W_ROWS
        counts[k] = np.bincount(w, minlength=WIN_PER_CORE)
        per_core.append((w, r, ids_sorted[s:e]))

    t_w = np.maximum(1, (counts.max(axis=0) + P - 1) // P)  # tiles per window
    n_tiles = int(t_w.sum())
    lcm = np.lcm(CHUNK_TILES, OH_BATCH)
    n_tiles = int(((n_tiles + lcm - 1) // lcm) * lcm)
    slot_base = np.zeros(WIN_PER_CORE + 1, dtype=np.int64)
    slot_base[1:] = np.cumsum(t_w * P)

    cores = []
    for k in range(N_CORES):
        w, r, ids = per_core[k]
        win_start = np.searchsorted(w, np.arange(WIN_PER_CORE))
        idx_in_win = np.arange(len(w)) - win_start[w]
        slots = slot_base[w] + idx_in_win
        rvals = np.full((n_tiles * P,), -1.0, dtype=np.float32)
        rvals[slots] = r.astype(np.float32)
        cores.append((slots, ids, rvals))
    return n_tiles, t_w, cores


def _build_program(n_tiles, t_w, dt):
    nc = bacc.Bacc("TRN2", target_bir_lowering=False, debug=False)
    n_chunks = n_tiles // CHUNK_TILES
    payload = nc.dram_tensor(
        "payload", [n_chunks, P, CHUNK_TILES * FEAT], dt, kind="ExternalInput")
    rvals = nc.dram_tensor("rvals", [P, n_tiles], dt, kind="ExternalInput")
    iota = nc.dram_tensor("iota", [P, OH_BATCH * W_ROWS], dt, kind="ExternalInput")
    out = nc.dram_tensor(
        "out", [P, (WIN_PER_CORE // W_PACK) * FEAT], mybir.dt.float32,
        kind="ExternalOutput")

    with tile.TileContext(nc) as tc:
        with tc.tile_pool(name="sbuf", bufs=1) as sb, \
             tc.tile_pool(name="chunk", bufs=3) as chunkp, \
             tc.tile_pool(name="oh", bufs=4) as ohp, \
             tc.tile_pool(name="psum", bufs=8, space="PSUM") as pp:
            iota_t = sb.tile([P, OH_BATCH, W_ROWS], dt)
            nc.sync.dma_start(out=iota_t[:, :, :], in_=iota.ap())
            rv_t = sb.tile([P, n_tiles], dt)
            nc.sync.dma_start(out=rv_t[:], in_=rvals.ap())
            # staging split into quarters (separate tiles) so an in-flight
            # flush DMA doesn't WAR-serialize later window copies
            n_w2 = WIN_PER_CORE // W_PACK
            qb = [0, n_w2 // 4, n_w2 // 2, (3 * n_w2) // 4, n_w2]
            stg = [sb.tile([P, (qb[q + 1] - qb[q]) * FEAT], mybir.dt.float32,
                           name=f"stg{q}", tag=f"stg{q}") for q in range(4)]

            chunks = []
            for c in range(n_chunks):
                buf = chunkp.tile([P, CHUNK_TILES * FEAT], dt)
                nc.sync.dma_start(out=buf[:], in_=payload.ap()[c])
                chunks.append(buf)

            oh_blocks = [None] * (n_tiles // OH_BATCH)

            def oh_block(b):
                if oh_blocks[b] is None:
                    blk = ohp.tile([P, OH_BATCH, W_ROWS], dt)
                    nc.vector.tensor_tensor(
                        out=blk[:, :, :], in0=iota_t[:, :, :],
                        in1=rv_t[:, b * OH_BATCH:(b + 1) * OH_BATCH]
                            .to_broadcast([P, OH_BATCH, W_ROWS]),
                        op=mybir.AluOpType.is_equal)
                    oh_blocks[b] = blk
                return oh_blocks[b]

            t = 0
            for w in range(WIN_PER_CORE):
                ps = pp.tile([W_ROWS, FEAT], mybir.dt.float32, space="PSUM")
                tw = int(t_w[w])
                for j in range(tw):
                    c, tc_ = divmod(t, CHUNK_TILES)
                    blk = oh_block(t // OH_BATCH)
                    nc.tensor.matmul(
                        out=ps[:], lhsT=blk[:, t % OH_BATCH, :],
                        rhs=chunks[c][:, tc_ * FEAT:(tc_ + 1) * FEAT],
                        start=(j == 0), stop=(j == tw - 1))
                    t += 1
                h, w2 = w % W_PACK, w // W_PACK
                q = next(i for i in range(4) if w2 < qb[i + 1])
                nc.scalar.copy(
                    out=stg[q][h * W_ROWS:(h + 1) * W_ROWS,
                              (w2 - qb[q]) * FEAT:(w2 - qb[q] + 1) * FEAT],
                    in_=ps[:])
                # flush each finished quarter so the out-write overlaps compute
                if (w + 1) == qb[q + 1] * W_PACK:
                    nc.scalar.dma_start(
                        out=out.ap()[:, qb[q] * FEAT:qb[q + 1] * FEAT],
                        in_=stg[q][:])
    nc.compile()
    return nc


def kernel(unary, binary, deltas, index1, index2):
    dt = mybir.dt.float16 if DTYPE == "fp16" else mybir.dt.float32
    npdt = np.float16 if DTYPE == "fp16" else np.float32

    n_tiles, t_w, cores = _plan(np.asarray(index1), np.asarray(index2))
    n_chunks = n_tiles // CHUNK_TILES

    deltas = np.asarray(deltas)
    src = np.concatenate(
        [deltas[:, :FEAT], deltas[:, FEAT:2 * FEAT]], axis=0).astype(npdt)

    iota_np = np.tile(np.arange(W_ROWS, dtype=npdt), (P, OH_BATCH))
    in_maps = []
    for k in range(N_CORES):
        slots, ids, rvals = cores[k]
        pay = np.zeros((n_tiles * P, FEAT), dtype=npdt)
        pay[slots] = src[ids]
        pay = np.ascontiguousarray(
            pay.reshape(n_chunks, CHUNK_TILES, P, FEAT)
               .transpose(0, 2, 1, 3)
               .reshape(n_chunks, P, CHUNK_TILES * FEAT))
        in_maps.append({
            "payload": pay,
            "rvals": np.ascontiguousarray(rvals.reshape(n_tiles, P).T).astype(npdt),
            "iota": iota_np,
        })

    nc = _build_program(n_tiles, t_w, dt)

    trace = bool(int(os.environ.get("GROUPBY_TRACE", "0")))
    if trace:
        import trn_prof
        trn_prof.install()
    res = run_bass_kernel_spmd(nc, in_maps, list(range(N_CORES)), trace=trace)
    if trace:
        kernel.last_exec_time_ns = res.exec_time_ns

    out = np.empty((N_CORES * ROWS_PER_CORE, FEAT), dtype=np.float32)
    for k in range(N_CORES):
        slab = res.results[k]["out"]  # [P, (WIN//W_PACK)*FEAT]
        # staging[h*W_ROWS + r, w2*FEAT + f] = row (w2*W_PACK + h)*W_ROWS + r
        out[k * ROWS_PER_CORE:(k + 1) * ROWS_PER_CORE] = (
            slab.reshape(W_PACK, W_ROWS, WIN_PER_CORE // W_PACK, FEAT)
                .transpose(2, 0, 1, 3)
                .reshape(ROWS_PER_CORE, FEAT))
    out = out[:N]

    b = np.ascontiguousarray(deltas[:, 2 * FEAT:])
    return out, b


# revision 11
# speedup vs baseline: 1.1255x; 1.0021x over previous
"""Trainium2 Bass kernel for nn_GroupBy (dual scatter-add segment sum).

reference:
    out = zeros([N, 128]); out[i1] += deltas[:, :128]; out[i2] += deltas[:, 128:256]
    b   = deltas[:, 256:320]
    return out, b

Strategy (8 NeuronCores):
  - Shard output rows across cores (12544 rows per core).
  - Host-side shard step: form 2E (target, source-row) pairs, order each
    core's pairs by target window (W_ROWS rows), pad windows to a shared
    static SPMD schedule.  Index math + one permutation pass only; every
    floating-point += happens on device.
  - Device: stream pair payload sequentially (full-BW DMAs), build one-hot
    selection matrices (DVE is_equal vs a static iota, OH_BATCH tiles per
    op), matmul-accumulate into PSUM per window (TensorE, fp32 PSUM).
    Copy finished windows into an SBUF staging buffer; one final DMA writes
    the core's output slab.
  - Host unshard: transpose window slabs back to row-major and concatenate.
"""
import os
import numpy as np

from concourse import bass, bacc, tile, mybir
from concourse.bass_utils import run_bass_kernel_spmd

N = 100000
E = 500000
FEAT = 128
P = 128
N_CORES = 8
ROWS_PER_CORE = 12544
W_ROWS = int(os.environ.get("GROUPBY_WROWS", "64"))      # rows per window
WIN_PER_CORE = ROWS_PER_CORE // W_ROWS
W_PACK = P // W_ROWS            # windows packed per 128 staging partitions
CHUNK_TILES = 32                # payload tiles (128 pairs each) per DMA chunk
OH_BATCH = 8                    # one-hot tiles built per DVE op
DTYPE = os.environ.get("GROUPBY_DTYPE", "fp16")


def _plan(i1, i2):
    """Host shard step: order (target, source) pairs by target window per core.

    Pure index math on int arrays.
    """
    t_all = np.concatenate([i1[:, 0], i2[:, 0]]).astype(np.int64)
    pair_ids = np.arange(2 * E, dtype=np.int64)
    order = np.argsort(t_all, kind="stable")
    t_sorted = t_all[order]
    ids_sorted = pair_ids[order]

    core_of = t_sorted // ROWS_PER_CORE          # contiguous runs, 0..7
    core_starts = np.searchsorted(core_of, np.arange(N_CORES + 1))

    per_core = []
    counts = np.zeros((N_CORES, WIN_PER_CORE), dtype=np.int64)
    for k in range(N_CORES):
        s, e = core_starts[k], core_starts[k + 1]
        t_loc = t_sorted[s:e] - k * ROWS_PER_CORE
        w = t_loc // W_ROWS
        r = t_loc % W_ROWS
        counts[k] = np.bincount(w, minlength=WIN_PER_CORE)
        per_core.append((w, r, ids_sorted[s:e]))

    t_w = np.maximum(1, (counts.max(axis=0) + P - 1) // P)  # tiles per window
    n_tiles = int(t_w.sum())
    lcm = np.lcm(CHUNK_TILES, OH_BATCH)
    n_tiles = int(((n_tiles + lcm - 1) // lcm) * lcm)
    slot_base = np.zeros(WIN_PER_CORE + 1, dtype=np.int64)
    slot_base[1:] = np.cumsum(t_w * P)

    cores = []
    for k in range(N_CORES):
        w, r, ids = per_core[k]
        win_start = np.searchsorted(w, np.arange(WIN_PER_CORE))
        idx_in_win = np.arange(len(w)) - win_start[w]
        slots = slot_base[w] + idx_in_win
        rvals = np.full((n_tiles * P,), -1.0, dtype=np.float32)
        rvals[slots] = r.astype(np.float32)
        cores.append((slots, ids, rvals))
    return n_tiles, t_w, cores


def _build_program(n_tiles, t_w, dt):
    nc = bacc.Bacc("TRN2", target_bir_lowering=False, debug=False)
    n_chunks = n_tiles // CHUNK_TILES
    payload = nc.dram_tensor(
        "payload", [n_chunks, P, CHUNK_TILES * FEAT], dt, kind="ExternalInput")
    rvals = nc.dram_tensor("rvals", [P, n_tiles], dt, kind="ExternalInput")
    iota = nc.dram_tensor("iota", [P, OH_BATCH * W_ROWS], dt, kind="ExternalInput")
    out = nc.dram_tensor(
        "out", [P, (WIN_PER_CORE // W_PACK) * FEAT], mybir.dt.float32,
        kind="ExternalOutput")

    with tile.TileContext(nc) as tc:
        with tc.tile_pool(name="sbuf", bufs=1) as sb, \
             tc.tile_pool(name="chunk", bufs=5) as chunkp, \
             tc.tile_pool(name="oh", bufs=6) as ohp, \
             tc.tile_pool(name="psum", bufs=8, space="PSUM") as pp:
            iota_t = sb.tile([P, OH_BATCH, W_ROWS], dt)
            nc.sync.dma_start(out=iota_t[:, :, :], in_=iota.ap())
            rv_t = sb.tile([P, n_tiles], dt)
            nc.sync.dma_start(out=rv_t[:], in_=rvals.ap())
            # staging split into quarters (separate tiles) so an in-flight
            # flush DMA doesn't WAR-serialize later window copies
            n_w2 = WIN_PER_CORE // W_PACK
            qb = [0, n_w2 // 4, n_w2 // 2, (3 * n_w2) // 4, n_w2]
            stg = [sb.tile([P, (qb[q + 1] - qb[q]) * FEAT], mybir.dt.float32,
                           name=f"stg{q}", tag=f"stg{q}") for q in range(4)]

            chunks = []
            for c in range(n_chunks):
                buf = chunkp.tile([P, CHUNK_TILES * FEAT], dt)
                nc.sync.dma_start(out=buf[:], in_=payload.ap()[c])
                chunks.append(buf)

            oh_blocks = [None] * (n_tiles // OH_BATCH)

            def oh_block(b):
                if oh_blocks[b] is None:
                    blk = ohp.tile([P, OH_BATCH, W_ROWS], dt)
                    nc.vector.tensor_tensor(
                        out=blk[:, :, :], in0=iota_t[:, :, :],
                        in1=rv_t[:, b * OH_BATCH:(b + 1) * OH_BATCH]
                            .to_broadcast([P, OH_BATCH, W_ROWS]),
                        op=mybir.AluOpType.is_equal)
                    oh_blocks[b] = blk
                return oh_blocks[b]

            t = 0
            for w in range(WIN_PER_CORE):
                ps = pp.tile([W_ROWS, FEAT], mybir.dt.float32, space="PSUM")
                tw = int(t_w[w])
                for j in range(tw):
                    c, tc_ = divmod(t, CHUNK_TILES)
                    blk = oh_block(t // OH_BATCH)
                    nc.tensor.matmul(
                        out=ps[:], lhsT=blk[:, t % OH_BATCH, :],
                        rhs=chunks[c][:, tc_ * FEAT:(tc_ + 1) * FEAT],
                        start=(j == 0), stop=(j == tw - 1))
                    t += 1
                h, w2 = w % W_PACK, w // W_PACK
                q = next(i for i in range(4) if w2 < qb[i + 1])
                nc.scalar.copy(
                    out=stg[q][h * W_ROWS:(h + 1) * W_ROWS,
                              (w2 - qb[q]) * FEAT:(w2 - qb[q] + 1) * FEAT],
                    in_=ps[:])
                # flush each finished quarter so the out-write overlaps compute
                if (w + 1) == qb[q + 1] * W_PACK:
                    nc.scalar.dma_start(
                        out=out.ap()[:, qb[q] * FEAT:qb[q + 1] * FEAT],
                        in_=stg[q][:])
    nc.compile()
    return nc


def kernel(unary, binary, deltas, index1, index2):
    dt = mybir.dt.float16 if DTYPE == "fp16" else mybir.dt.float32
    npdt = np.float16 if DTYPE == "fp16" else np.float32

    n_tiles, t_w, cores = _plan(np.asarray(index1), np.asarray(index2))
    n_chunks = n_tiles // CHUNK_TILES

    deltas = np.asarray(deltas)
    src = np.concatenate(
        [deltas[:, :FEAT], deltas[:, FEAT:2 * FEAT]], axis=0).astype(npdt)

    iota_np = np.tile(np.arange(W_ROWS, dtype=npdt), (P, OH_BATCH))
    in_maps = []
    for k in range(N_CORES):
        slots, ids, rvals = cores[k]
        pay = np.zeros((n_tiles * P, FEAT), dtype=npdt)
        pay[slots] = src[ids]
        pay = np.ascontiguousarray(
            pay.reshape(n_chunks, CHUNK_TILES, P, FEAT)
               .transpose(0, 2, 1, 3)
               .reshape(n_chunks, P, CHUNK_TILES * FEAT))
        in_maps.append({
            "payload": pay,
            "rvals": np.ascontiguousarray(rvals.reshape(n_tiles, P).T).astype(npdt),
            "iota": iota_np,
        })

    nc = _build_program(n_tiles, t_w, dt)

    trace = bool(int(os.environ.get("GROUPBY_TRACE", "0")))
    if trace:
        import trn_prof
        trn_prof.install()
    res = run_bass_kernel_spmd(nc, in_maps, list(range(N_CORES)), trace=trace)
    if trace:
        kernel.last_exec_time_ns = res.exec_time_ns

    out = np.empty((N_CORES * ROWS_PER_CORE, FEAT), dtype=np.float32)
    for k in range(N_CORES):
        slab = res.results[k]["out"]  # [P, (WIN//W_PACK)*FEAT]
        # staging[h*W_ROWS + r, w2*FEAT + f] = row (w2*W_PACK + h)*W_ROWS + r
        out[k * ROWS_PER_CORE:(k + 1) * ROWS_PER_CORE] = (
            slab.reshape(W_PACK, W_ROWS, WIN_PER_CORE // W_PACK, FEAT)
                .transpose(2, 0, 1, 3)
                .reshape(ROWS_PER_CORE, FEAT))
    out = out[:N]

    b = np.ascontiguousarray(deltas[:, 2 * FEAT:])
    return out, b
